# revision 34
# baseline (speedup 1.0000x reference)
"""CombinedCSA (channel+spatial attention) Trainium2 Bass kernel, bf16.

Sharding: data-parallel over batch. 16 images / 8 cores = 2 images per core.
Weights (fc1/fc2/conv) replicated, pre-transposed host-side. x is downcast
to bf16 host-side; output is stored bf16 and upcast host-side (rel-err
budget 2e-2 >> bf16 rounding).

Per-image stages, software-pipelined 3 deep (slot t interleaves, chunk by
chunk, stage A of image t with stage C of image t-1 and stage E of image
t-2 so every engine queue sees a round-robin mix):
  A. stream 8 double-chunks [128, 4096] per half in; DVE running-fold
     channel-max; PE matmul w1 @ x into PSUM (= fc1 of the mean, folded)
  B. MLP: PE w1@chmax + ACT relu(+mean bias) + PE fc2 + ACT sigmoid -> s
  C. scale halves in place (DVE tensor_scalar / ACT copy-scale), DVE
     max-combine halves, gpsimd partition_all_reduce(max) -> spatial-max
     row, row DMA into [h, w] stat tile; PE sliding-ones matmuls -> spatial
     sum in [h, w] layout
  D. 7x7 conv as 14 banded matmuls on PE (bands shift along w, [h, w]
     layout), ACT sigmoid -> attn
  E. attn row DMA, PE outer-product broadcast (ones x row) to PSUM, ACT
     evac to bf16, DVE multiply in place, store
"""

import numpy as np
import ml_dtypes
from contextlib import ExitStack

import concourse.bass as bass
import concourse.tile as tile
from concourse import bacc, mybir, bass_isa
from concourse.bass_utils import run_bass_kernel_spmd

F32 = mybir.dt.float32
BF16 = mybir.dt.bfloat16
AF = mybir.ActivationFunctionType
ALU = mybir.AluOpType

# Problem constants (hardcoded; see spec)
B, C, H, W = 16, 256, 128, 128
HW = H * W          # 16384
R = 16              # Cr = C // 16
NCORES = 8
BLOC = B // NCORES  # 2 images per core
NH = 2              # channel halves of 128
P = 128
FCH = 2048          # hw elements per chunk (16 h-rows)
NCH = HW // FCH     # 8 chunks per image
HROWS = FCH // W    # 16 h-rows per chunk
FH = 1024           # chmax fold width


def make_pools(ctx, tc):
    pools = {}
    pools["xp"] = ctx.enter_context(tc.tile_pool(name="xp", bufs=18))
    pools["mp"] = ctx.enter_context(tc.tile_pool(name="mp", bufs=3))
    pools["mrp"] = ctx.enter_context(tc.tile_pool(name="mrp", bufs=2))
    pools["bcp"] = ctx.enter_context(tc.tile_pool(name="bcp", bufs=2))
    pools["stat"] = ctx.enter_context(tc.tile_pool(name="stat", bufs=2))
    pools["cons"] = ctx.enter_context(tc.tile_pool(name="cons", bufs=1))
    pools["fc1p"] = ctx.enter_context(
        tc.tile_pool(name="fc1p", bufs=1, space="PSUM"))
    pools["svp"] = ctx.enter_context(
        tc.tile_pool(name="svp", bufs=1, space="PSUM"))
    pools["bcps"] = ctx.enter_context(
        tc.tile_pool(name="bcps", bufs=2, space="PSUM"))
    pools["convp"] = ctx.enter_context(
        tc.tile_pool(name="convp", bufs=1, space="PSUM"))
    pools["mlpp"] = ctx.enter_context(
        tc.tile_pool(name="mlpp", bufs=1, space="PSUM"))
    return pools


def load_consts(tc, pools, w1t_d, w2t_d, bands_d, oh_d):
    nc = tc.nc
    cons = pools["cons"]
    w1t_sb = cons.tile([P, NH * R], BF16)          # [128, 32]: w_fc1.T halves
    for h in range(NH):
        nc.sync.dma_start(out=w1t_sb[:, h * R:(h + 1) * R],
                          in_=w1t_d[h * P:(h + 1) * P, :])
    w2t_sb = cons.tile([R, C], BF16)               # [16, 256] = w_fc2.T
    nc.sync.dma_start(out=w2t_sb[:], in_=w2t_d[:])
    bands_sb = cons.tile([P, 14 * P], BF16)        # [h, (c*7+j, h')]
    nc.sync.dma_start(out=bands_sb[:].rearrange("p (c w) -> p c w", c=14),
                      in_=bands_d.transpose([1, 0, 2]))
    oh_sb = cons.tile([P, 2 * W], BF16)            # all-ones column at index W
    nc.sync.dma_start(out=oh_sb[:], in_=oh_d[:])
    ones_sb = cons.tile([1, P], BF16)
    nc.vector.memset(ones_sb[:], 1.0)
    return dict(w1t_sb=w1t_sb, w2t_sb=w2t_sb, bands_sb=bands_sb,
                oh_sb=oh_sb, ones_sb=ones_sb)


class ImageState:
    """Per-image tiles and APs threaded between pipeline stages."""

    def __init__(self, i, b):
        self.i = i          # global image index (name uniquifier)
        self.b = b          # batch slot on this core
        self.xtt = [[None] * (NCH // 2) for _ in range(NH)]
        self.xt = [[None] * NCH for _ in range(NH)]
        self.runm = [None] * NH
        self.fc1_ps = None
        self.s_f32 = []
        self.smaxHW = None
        self.savgHW = None
        self.sv_ps = None
        self.attn = None


def emit_A_chunk(tc, pools, consts, st, x_d, k, skip):
    """Load (on even k) + channel-stat work for sub-chunk k."""
    nc = tc.nc
    i, b = st.i, st.b
    kk, sub = divmod(k, 2)
    if sub == 0:
        for h in range(NH):
            t = pools["xp"].tile([P, 2 * FCH], BF16, name=f"x{i}_{h}{kk}",
                                 tag="x")
            st.xtt[h][kk] = t
            st.xt[h][2 * kk] = t[:, 0:FCH]
            st.xt[h][2 * kk + 1] = t[:, FCH:2 * FCH]
            nc.sync.dma_start(
                out=t[:],
                in_=x_d[b, h * P:(h + 1) * P,
                        2 * kk * FCH:(2 * kk + 2) * FCH])
    if k == 0:
        for h in range(NH):
            st.runm[h] = pools["stat"].tile([P, FH], BF16, name=f"runm{i}_{h}",
                                            tag=f"runm{h}")
        st.fc1_ps = pools["fc1p"].tile([R, 512], F32, name=f"fc1ps{i}",
                                       tag="fc1")
    if "chpool" in skip:
        return
    for h in range(NH):
        tv = st.xt[h][k]
        if k == 0:
            nc.vector.tensor_max(st.runm[h][:], tv[:, 0:FH], tv[:, FH:FCH])
        else:
            nc.vector.tensor_max(st.runm[h][:], st.runm[h][:], tv[:, 0:FH])
            nc.vector.tensor_max(st.runm[h][:], st.runm[h][:], tv[:, FH:FCH])
        for j in range(4):
            nc.tensor.matmul(
                out=st.fc1_ps[:],
                lhsT=consts["w1t_sb"][:, h * R:(h + 1) * R],
                rhs=tv[:, j * 512:(j + 1) * 512],
                start=(k == 0 and h == 0 and j == 0),
                stop=(k == NCH - 1 and h == NH - 1 and j == 3),
                skip_group_check=True)


def emit_B(tc, pools, consts, st, skip):
    nc = tc.nc
    i = st.i
    stat = pools["stat"]
    mlp_ps = pools["mlpp"].tile([P, 4], F32, name=f"mlps{i}", tag="mlp")
    z_ps = mlp_ps[0:R, 0:1]
    mean16 = stat.tile([R, 1], F32, name=f"mean{i}", tag="mean")
    trash = stat.tile([R, 512], BF16, name=f"trash{i}", tag="trash")
    if "chpool" in skip:
        nc.vector.memset(mean16[:], 0.1)
    else:
        nc.scalar.activation(out=trash[:], in_=st.fc1_ps[:], func=AF.Copy,
                             scale=1.0 / HW, accum_out=mean16[:])
    for h in range(NH):
        cm = stat.tile([P, 1], BF16, name=f"cm{i}_{h}", tag=f"cm{h}")
        if "chpool" in skip:
            nc.vector.memset(cm[:], 0.5)
        else:
            nc.vector.tensor_reduce(out=cm[:], in_=st.runm[h][:],
                                    axis=mybir.AxisListType.X, op=ALU.max)
        nc.tensor.matmul(out=z_ps, lhsT=consts["w1t_sb"][:, h * R:(h + 1) * R],
                         rhs=cm[:], start=(h == 0), stop=(h == NH - 1),
                         skip_group_check=True)
    zr = stat.tile([R, 1], BF16, name=f"zr{i}", tag="zr")
    nc.scalar.activation(out=zr[:], in_=z_ps, func=AF.Relu,
                         bias=mean16[:, 0:1])
    st.s_f32 = []
    for h in range(NH):
        l_ps = mlp_ps[:, 1 + h:2 + h]
        nc.tensor.matmul(out=l_ps, lhsT=consts["w2t_sb"][:, h * P:(h + 1) * P],
                         rhs=zr[:], start=True, stop=True,
                         skip_group_check=True)
        sc = stat.tile([P, 1], F32, name=f"s{i}_{h}", tag=f"s{h}")
        nc.scalar.activation(out=sc[:], in_=l_ps, func=AF.Sigmoid)
        st.s_f32.append(sc)
    st.smaxHW = stat.tile([P, W], BF16, name=f"smax{i}", tag="smax")
    st.savgHW = stat.tile([P, W], BF16, name=f"savg{i}", tag="savg")
    st.sv_ps = pools["svp"].tile([P, W], F32, name=f"svps{i}", tag="sv")
    if "trans" in skip:
        nc.vector.memset(st.smaxHW[:], 0.25)
    if "savg" in skip:
        nc.vector.memset(st.savgHW[:], 0.25)


def emit_C_chunk(tc, pools, consts, st, k, skip):
    nc = tc.nc
    i = st.i
    if "scale" not in skip:
        # scale both halves in place: DVE for h0 (4x mode), ACT for h1
        nc.vector.tensor_scalar_mul(st.xt[0][k], st.xt[0][k],
                                    st.s_f32[0][:, 0:1])
        nc.scalar.activation(out=st.xt[1][k], in_=st.xt[1][k],
                             func=AF.Copy, scale=st.s_f32[1][:, 0:1])
    if "trans" not in skip:
        m = pools["mp"].tile([P, FCH], BF16, name=f"m{i}_{k}", tag="m")
        nc.vector.tensor_max(m[:], st.xt[0][k], st.xt[1][k])
        mr = pools["mrp"].tile([P, FCH], BF16, name=f"mr{i}_{k}", tag="mr")
        nc.gpsimd.partition_all_reduce(mr[:], m[:], channels=P,
                                       reduce_op=bass_isa.ReduceOp.max)
        nc.sync.dma_start(
            out=st.smaxHW[k * HROWS:(k + 1) * HROWS, :],
            in_=mr[0:1, :].rearrange("p (h w) -> p h w", h=HROWS))
    if "savg" not in skip:
        for j in range(HROWS):
            q = k * HROWS + j
            for h in range(NH):
                nc.tensor.matmul(
                    out=st.sv_ps[:],
                    lhsT=consts["oh_sb"][:, W - q:2 * W - q],
                    rhs=st.xt[h][k][:, j * W:(j + 1) * W],
                    start=(k == 0 and j == 0 and h == 0),
                    stop=(k == NCH - 1 and j == HROWS - 1 and h == NH - 1),
                    skip_group_check=True)


def emit_D(tc, pools, consts, st, skip):
    nc = tc.nc
    i = st.i
    if "savg" not in skip:
        nc.scalar.activation(out=st.savgHW[:], in_=st.sv_ps[:], func=AF.Copy)
    st.attn = pools["stat"].tile([P, W], BF16, name=f"attn{i}", tag="attn")
    if "conv" in skip:
        return
    conv_ps = pools["convp"].tile([P, W], F32, name=f"convps{i}", tag="conv")
    mms = []
    for c, stt in ((0, st.smaxHW), (1, st.savgHW)):
        for j in range(7):
            lo = max(0, 3 - j)
            hi = min(W, W + 3 - j)
            mms.append((c, j, lo, hi, stt))
    mms.sort(key=lambda mm: (mm[1] != 3 or mm[0] != 0))
    for n, (c, j, lo, hi, stt) in enumerate(mms):
        nc.tensor.matmul(
            out=conv_ps[:, lo:hi],
            lhsT=consts["bands_sb"][:, (c * 7 + j) * P:(c * 7 + j + 1) * P],
            rhs=stt[:, lo + j - 3:hi + j - 3],
            start=(n == 0), stop=(n == len(mms) - 1),
            skip_group_check=True)
    nc.scalar.activation(out=st.attn[:], in_=conv_ps[:], func=AF.Sigmoid)


def emit_E_chunk(tc, pools, consts, st, out_d, k, skip):
    nc = tc.nc
    i, b = st.i, st.b
    if "conv" not in skip and "final" not in skip:
        row = pools["stat"].tile([1, FCH], BF16, name=f"row{i}_{k}",
                                 tag="row", bufs=3)
        nc.sync.dma_start(
            out=row[:].rearrange("p (h w) -> p h w", h=HROWS),
            in_=st.attn[k * HROWS:(k + 1) * HROWS, :])
        bc_sb = pools["bcp"].tile([P, FCH], BF16, name=f"bc{i}_{k}", tag="bc")
        for j in range(2):
            bc_ps = pools["bcps"].tile([P, FH], F32, name=f"bcps{i}_{k}{j}",
                                       tag="bcps")
            for jj in range(2):
                nc.tensor.matmul(
                    out=bc_ps[:, jj * 512:(jj + 1) * 512],
                    lhsT=consts["ones_sb"][:],
                    rhs=row[0:1, j * FH + jj * 512:j * FH + (jj + 1) * 512],
                    start=True, stop=True, skip_group_check=True)
            nc.scalar.activation(out=bc_sb[:, j * FH:(j + 1) * FH],
                                 in_=bc_ps[:], func=AF.Copy)
        for h in range(NH):
            nc.vector.tensor_mul(st.xt[h][k], st.xt[h][k], bc_sb[:])
    if k % 2 == 1:
        for h in range(NH):
            nc.gpsimd.dma_start(
                out=out_d[b, h * P:(h + 1) * P, (k - 1) * FCH:(k + 1) * FCH],
                in_=st.xtt[h][k // 2][:])


def emit_pipeline(tc, pools, consts, out_d, x_d, n_images, skip=frozenset()):
    """3-deep software pipeline over images: slot t runs A(t) | C(t-1) |
    E(t-2) chunk-interleaved, then D(t-1), then B(t)."""
    prev = None
    for t in range(n_images + 1):
        st = ImageState(t, t % BLOC) if t < n_images else None
        for k in range(NCH):
            if st is not None:
                emit_A_chunk(tc, pools, consts, st, x_d, k, skip)
            if prev is not None:
                emit_E_chunk(tc, pools, consts, prev, out_d, k, skip)
        if st is not None:
            emit_B(tc, pools, consts, st, skip)
            for k in range(NCH):
                emit_C_chunk(tc, pools, consts, st, k, skip)
            emit_D(tc, pools, consts, st, skip)
        prev = st


def _build_nc(reps: int = 1, skip=frozenset()):
    nc = bacc.Bacc("TRN2", target_bir_lowering=False, debug=False,
                   num_devices=NCORES)
    x_d = nc.dram_tensor("x", [BLOC, C, HW], BF16, kind="ExternalInput").ap()
    w1t_d = nc.dram_tensor("w1t", [C, R], BF16, kind="ExternalInput").ap()
    w2t_d = nc.dram_tensor("w2t", [R, C], BF16, kind="ExternalInput").ap()
    bands_d = nc.dram_tensor("bands", [14, W, W], BF16,
                             kind="ExternalInput").ap()
    oh_d = nc.dram_tensor("oh", [P, 2 * W], BF16, kind="ExternalInput").ap()
    out_d = nc.dram_tensor("out", [BLOC, C, HW], BF16,
                           kind="ExternalOutput").ap()
    with tile.TileContext(nc) as tc:
        with ExitStack() as ctx:
            pools = make_pools(ctx, tc)
            consts = load_consts(tc, pools, w1t_d, w2t_d, bands_d, oh_d)
            emit_pipeline(tc, pools, consts, out_d, x_d, reps * BLOC,
                          skip=skip)
    nc.compile()
    return nc


_NC_CACHE = None


def _get_nc():
    global _NC_CACHE
    if _NC_CACHE is None:
        _NC_CACHE = _build_nc()
    return _NC_CACHE


def build_bands(w_conv):
    """[14, H, H] band matrices for the [h, w] conv formulation:
    bands[c*7+j][h, h'] = w_conv[0, c, h-h'+3, j]; avg channel folded 1/C."""
    w_conv = np.asarray(w_conv, np.float32)
    bands = np.zeros((2, 7, H, H), np.float32)
    for c in range(2):
        for j in range(7):
            for i in range(7):
                bands[c, j] += w_conv[0, c, i, j] * np.eye(H, k=3 - i,
                                                           dtype=np.float32)
    bands[1] /= C
    return bands.reshape(14, H, H)


def build_onehot():
    oh = np.zeros((P, 2 * W), np.float32)
    oh[:, W] = 1.0
    return oh


def make_in_maps(x, w_fc1, w_fc2, w_conv):
    bf = ml_dtypes.bfloat16
    x = np.ascontiguousarray(np.asarray(x)).astype(bf)
    w1t = np.ascontiguousarray(np.asarray(w_fc1, np.float32).T).astype(bf)
    w2t = np.ascontiguousarray(np.asarray(w_fc2, np.float32).T).astype(bf)
    bands = build_bands(w_conv).astype(bf)
    oh = build_onehot().astype(bf)
    xr = x.reshape(NCORES, BLOC, C, HW)
    return [{"x": np.ascontiguousarray(xr[i]), "w1t": w1t, "w2t": w2t,
             "bands": bands, "oh": oh} for i in range(NCORES)]


def kernel(x, w_fc1, w_fc2, w_conv):
    nc = _get_nc()
    in_maps = make_in_maps(x, w_fc1, w_fc2, w_conv)
    res = run_bass_kernel_spmd(nc, in_maps, list(range(NCORES)))
    out = np.stack([np.asarray(res.results[i]["out"]) for i in range(NCORES)])
    return out.reshape(B, C, H, W).astype(np.float32)


# revision 35
# speedup vs baseline: 2.8287x; 2.8287x over previous
"""CombinedCSA (channel+spatial attention) Trainium2 Bass kernel, bf16.

Sharding: data-parallel over batch. 16 images / 8 cores = 2 images per core.
Weights (fc1/fc2/conv) replicated, pre-transposed host-side. x is downcast
to bf16 host-side; output is stored bf16 and upcast host-side (rel-err
budget 2e-2 >> bf16 rounding).

Per-image stages, software-pipelined 3 deep (slot t interleaves, chunk by
chunk, stage A of image t with stage C of image t-1 and stage E of image
t-2 so every engine queue sees a round-robin mix):
  A. stream 8 double-chunks [128, 4096] per half in; DVE running-fold
     channel-max; PE matmul w1 @ x into PSUM (= fc1 of the mean, folded)
  B. MLP: PE w1@chmax + ACT relu(+mean bias) + PE fc2 + ACT sigmoid -> s
  C. scale halves in place (DVE tensor_scalar / ACT copy-scale), DVE
     max-combine halves, gpsimd partition_all_reduce(max) -> spatial-max
     row, row DMA into [h, w] stat tile; PE sliding-ones matmuls -> spatial
     sum in [h, w] layout
  D. 7x7 conv as 14 banded matmuls on PE (bands shift along w, [h, w]
     layout), ACT sigmoid -> attn
  E. attn row DMA, PE outer-product broadcast (ones x row) to PSUM, ACT
     evac to bf16, DVE multiply in place, store
"""

import numpy as np
import ml_dtypes
from contextlib import ExitStack

import concourse.bass as bass
import concourse.tile as tile
from concourse import bacc, mybir, bass_isa
from concourse.bass_utils import run_bass_kernel_spmd

F32 = mybir.dt.float32
BF16 = mybir.dt.bfloat16
AF = mybir.ActivationFunctionType
ALU = mybir.AluOpType

# Problem constants (hardcoded; see spec)
B, C, H, W = 16, 256, 128, 128
HW = H * W          # 16384
R = 16              # Cr = C // 16
NCORES = 8
BLOC = B // NCORES  # 2 images per core
NH = 2              # channel halves of 128
P = 128
FCH = 2048          # hw elements per chunk (16 h-rows)
NCH = HW // FCH     # 8 chunks per image
HROWS = FCH // W    # 16 h-rows per chunk
FH = 1024           # chmax fold width


def make_pools(ctx, tc):
    pools = {}
    pools["xp"] = ctx.enter_context(tc.tile_pool(name="xp", bufs=18))
    pools["mp"] = ctx.enter_context(tc.tile_pool(name="mp", bufs=3))
    pools["mrp"] = ctx.enter_context(tc.tile_pool(name="mrp", bufs=2))
    pools["bcp"] = ctx.enter_context(tc.tile_pool(name="bcp", bufs=2))
    pools["stat"] = ctx.enter_context(tc.tile_pool(name="stat", bufs=2))
    pools["cons"] = ctx.enter_context(tc.tile_pool(name="cons", bufs=1))
    pools["fc1p"] = ctx.enter_context(
        tc.tile_pool(name="fc1p", bufs=1, space="PSUM"))
    pools["svp"] = ctx.enter_context(
        tc.tile_pool(name="svp", bufs=1, space="PSUM"))
    pools["bcps"] = ctx.enter_context(
        tc.tile_pool(name="bcps", bufs=2, space="PSUM"))
    pools["convp"] = ctx.enter_context(
        tc.tile_pool(name="convp", bufs=1, space="PSUM"))
    pools["mlpp"] = ctx.enter_context(
        tc.tile_pool(name="mlpp", bufs=1, space="PSUM"))
    return pools


def load_consts(tc, pools, w1t_d, w2t_d, bands_d, oh_d):
    nc = tc.nc
    cons = pools["cons"]
    w1t_sb = cons.tile([P, NH * R], BF16)          # [128, 32]: w_fc1.T halves
    for h in range(NH):
        nc.sync.dma_start(out=w1t_sb[:, h * R:(h + 1) * R],
                          in_=w1t_d[h * P:(h + 1) * P, :])
    w2t_sb = cons.tile([R, C], BF16)               # [16, 256] = w_fc2.T
    nc.sync.dma_start(out=w2t_sb[:], in_=w2t_d[:])
    bands_sb = cons.tile([P, 14 * P], BF16)        # [h, (c*7+j, h')]
    nc.sync.dma_start(out=bands_sb[:].rearrange("p (c w) -> p c w", c=14),
                      in_=bands_d.transpose([1, 0, 2]))
    oh_sb = cons.tile([P, 2 * W], BF16)            # all-ones column at index W
    nc.sync.dma_start(out=oh_sb[:], in_=oh_d[:])
    ones_sb = cons.tile([1, P], BF16)
    nc.vector.memset(ones_sb[:], 1.0)
    return dict(w1t_sb=w1t_sb, w2t_sb=w2t_sb, bands_sb=bands_sb,
                oh_sb=oh_sb, ones_sb=ones_sb)


class ImageState:
    """Per-image tiles and APs threaded between pipeline stages."""

    def __init__(self, i, b):
        self.i = i          # global image index (name uniquifier)
        self.b = b          # batch slot on this core
        self.xtt = [[None] * (NCH // 2) for _ in range(NH)]
        self.xt = [[None] * NCH for _ in range(NH)]
        self.runm = [None] * NH
        self.fc1_ps = None
        self.s_f32 = []
        self.smaxHW = None
        self.savgHW = None
        self.sv_ps = None
        self.attn = None


def emit_A_chunk(tc, pools, consts, st, x_d, k, skip):
    """Load (on even k) + channel-stat work for sub-chunk k."""
    nc = tc.nc
    i, b = st.i, st.b
    kk, sub = divmod(k, 2)
    if sub == 0:
        for h in range(NH):
            t = pools["xp"].tile([P, 2 * FCH], BF16, name=f"x{i}_{h}{kk}",
                                 tag="x")
            st.xtt[h][kk] = t
            st.xt[h][2 * kk] = t[:, 0:FCH]
            st.xt[h][2 * kk + 1] = t[:, FCH:2 * FCH]
            nc.sync.dma_start(
                out=t[:],
                in_=x_d[b, h * P:(h + 1) * P,
                        2 * kk * FCH:(2 * kk + 2) * FCH])
    if k == 0:
        for h in range(NH):
            st.runm[h] = pools["stat"].tile([P, FH], BF16, name=f"runm{i}_{h}",
                                            tag=f"runm{h}")
        st.fc1_ps = pools["fc1p"].tile([R, 512], F32, name=f"fc1ps{i}",
                                       tag="fc1")
    if "chpool" in skip:
        return
    for h in range(NH):
        tv = st.xt[h][k]
        if k == 0:
            nc.vector.tensor_max(st.runm[h][:], tv[:, 0:FH], tv[:, FH:FCH])
        else:
            nc.vector.tensor_max(st.runm[h][:], st.runm[h][:], tv[:, 0:FH])
            nc.vector.tensor_max(st.runm[h][:], st.runm[h][:], tv[:, FH:FCH])
        for j in range(4):
            nc.tensor.matmul(
                out=st.fc1_ps[:],
                lhsT=consts["w1t_sb"][:, h * R:(h + 1) * R],
                rhs=tv[:, j * 512:(j + 1) * 512],
                start=(k == 0 and h == 0 and j == 0),
                stop=(k == NCH - 1 and h == NH - 1 and j == 3),
                skip_group_check=True)


def emit_B(tc, pools, consts, st, skip):
    nc = tc.nc
    i = st.i
    stat = pools["stat"]
    mlp_ps = pools["mlpp"].tile([P, 4], F32, name=f"mlps{i}", tag="mlp")
    z_ps = mlp_ps[0:R, 0:1]
    mean16 = stat.tile([R, 1], F32, name=f"mean{i}", tag="mean")
    trash = stat.tile([R, 512], BF16, name=f"trash{i}", tag="trash")
    if "chpool" in skip:
        nc.vector.memset(mean16[:], 0.1)
    else:
        nc.scalar.activation(out=trash[:], in_=st.fc1_ps[:], func=AF.Copy,
                             scale=1.0 / HW, accum_out=mean16[:])
    for h in range(NH):
        cm = stat.tile([P, 1], BF16, name=f"cm{i}_{h}", tag=f"cm{h}")
        if "chpool" in skip:
            nc.vector.memset(cm[:], 0.5)
        else:
            nc.vector.tensor_reduce(out=cm[:], in_=st.runm[h][:],
                                    axis=mybir.AxisListType.X, op=ALU.max)
        nc.tensor.matmul(out=z_ps, lhsT=consts["w1t_sb"][:, h * R:(h + 1) * R],
                         rhs=cm[:], start=(h == 0), stop=(h == NH - 1),
                         skip_group_check=True)
    zr = stat.tile([R, 1], BF16, name=f"zr{i}", tag="zr")
    nc.scalar.activation(out=zr[:], in_=z_ps, func=AF.Relu,
                         bias=mean16[:, 0:1])
    st.s_f32 = []
    for h in range(NH):
        l_ps = mlp_ps[:, 1 + h:2 + h]
        nc.tensor.matmul(out=l_ps, lhsT=consts["w2t_sb"][:, h * P:(h + 1) * P],
                         rhs=zr[:], start=True, stop=True,
                         skip_group_check=True)
        sc = stat.tile([P, 1], F32, name=f"s{i}_{h}", tag=f"s{h}")
        nc.scalar.activation(out=sc[:], in_=l_ps, func=AF.Sigmoid)
        st.s_f32.append(sc)
    st.smaxHW = stat.tile([P, W], BF16, name=f"smax{i}", tag="smax")
    st.savgHW = stat.tile([P, W], BF16, name=f"savg{i}", tag="savg")
    st.sv_ps = pools["svp"].tile([P, W], F32, name=f"svps{i}", tag="sv")
    if "trans" in skip:
        nc.vector.memset(st.smaxHW[:], 0.25)
    if "savg" in skip:
        nc.vector.memset(st.savgHW[:], 0.25)


def emit_C_chunk(tc, pools, consts, st, k, skip):
    nc = tc.nc
    i = st.i
    if "scale" not in skip:
        # scale both halves in place: DVE for h0 (4x mode), ACT for h1
        nc.vector.tensor_scalar_mul(st.xt[0][k], st.xt[0][k],
                                    st.s_f32[0][:, 0:1])
        nc.scalar.activation(out=st.xt[1][k], in_=st.xt[1][k],
                             func=AF.Copy, scale=st.s_f32[1][:, 0:1])
    if "trans" not in skip:
        m = pools["mp"].tile([P, FCH], BF16, name=f"m{i}_{k}", tag="m")
        nc.vector.tensor_max(m[:], st.xt[0][k], st.xt[1][k])
        mr = pools["mrp"].tile([P, FCH], BF16, name=f"mr{i}_{k}", tag="mr")
        nc.gpsimd.partition_all_reduce(mr[:], m[:], channels=P,
                                       reduce_op=bass_isa.ReduceOp.max)
        nc.sync.dma_start(
            out=st.smaxHW[k * HROWS:(k + 1) * HROWS, :],
            in_=mr[0:1, :].rearrange("p (h w) -> p h w", h=HROWS))
    if "savg" not in skip:
        for j in range(HROWS):
            q = k * HROWS + j
            for h in range(NH):
                nc.tensor.matmul(
                    out=st.sv_ps[:],
                    lhsT=consts["oh_sb"][:, W - q:2 * W - q],
                    rhs=st.xt[h][k][:, j * W:(j + 1) * W],
                    start=(k == 0 and j == 0 and h == 0),
                    stop=(k == NCH - 1 and j == HROWS - 1 and h == NH - 1),
                    skip_group_check=True)


def emit_D(tc, pools, consts, st, skip):
    nc = tc.nc
    i = st.i
    if "savg" not in skip:
        nc.scalar.activation(out=st.savgHW[:], in_=st.sv_ps[:], func=AF.Copy)
    st.attn = pools["stat"].tile([P, W], BF16, name=f"attn{i}", tag="attn")
    if "conv" in skip:
        return
    conv_ps = pools["convp"].tile([P, W], F32, name=f"convps{i}", tag="conv")
    mms = []
    for c, stt in ((0, st.smaxHW), (1, st.savgHW)):
        for j in range(7):
            lo = max(0, 3 - j)
            hi = min(W, W + 3 - j)
            mms.append((c, j, lo, hi, stt))
    mms.sort(key=lambda mm: (mm[1] != 3 or mm[0] != 0))
    for n, (c, j, lo, hi, stt) in enumerate(mms):
        nc.tensor.matmul(
            out=conv_ps[:, lo:hi],
            lhsT=consts["bands_sb"][:, (c * 7 + j) * P:(c * 7 + j + 1) * P],
            rhs=stt[:, lo + j - 3:hi + j - 3],
            start=(n == 0), stop=(n == len(mms) - 1),
            skip_group_check=True)
    nc.scalar.activation(out=st.attn[:], in_=conv_ps[:], func=AF.Sigmoid)


def emit_E_chunk(tc, pools, consts, st, out_d, k, skip):
    nc = tc.nc
    i, b = st.i, st.b
    if "conv" not in skip and "final" not in skip:
        row = pools["stat"].tile([1, FCH], BF16, name=f"row{i}_{k}",
                                 tag="row", bufs=3)
        nc.sync.dma_start(
            out=row[:].rearrange("p (h w) -> p h w", h=HROWS),
            in_=st.attn[k * HROWS:(k + 1) * HROWS, :])
        bc_sb = pools["bcp"].tile([P, FCH], BF16, name=f"bc{i}_{k}", tag="bc")
        for j in range(2):
            bc_ps = pools["bcps"].tile([P, FH], F32, name=f"bcps{i}_{k}{j}",
                                       tag="bcps")
            for jj in range(2):
                nc.tensor.matmul(
                    out=bc_ps[:, jj * 512:(jj + 1) * 512],
                    lhsT=consts["ones_sb"][:],
                    rhs=row[0:1, j * FH + jj * 512:j * FH + (jj + 1) * 512],
                    start=True, stop=True, skip_group_check=True)
            nc.scalar.activation(out=bc_sb[:, j * FH:(j + 1) * FH],
                                 in_=bc_ps[:], func=AF.Copy)
        for h in range(NH):
            nc.vector.tensor_mul(st.xt[h][k], st.xt[h][k], bc_sb[:])
    if k % 2 == 1:
        for h in range(NH):
            nc.scalar.dma_start(
                out=out_d[b, h * P:(h + 1) * P, (k - 1) * FCH:(k + 1) * FCH],
                in_=st.xtt[h][k // 2][:])


def emit_pipeline(tc, pools, consts, out_d, x_d, n_images, skip=frozenset()):
    """3-deep software pipeline over images: slot t runs A(t) | C(t-1) |
    E(t-2) chunk-interleaved, then D(t-1), then B(t)."""
    prev = None
    for t in range(n_images + 1):
        st = ImageState(t, t % BLOC) if t < n_images else None
        for k in range(NCH):
            if st is not None:
                emit_A_chunk(tc, pools, consts, st, x_d, k, skip)
            if prev is not None:
                emit_E_chunk(tc, pools, consts, prev, out_d, k, skip)
        if st is not None:
            emit_B(tc, pools, consts, st, skip)
            for k in range(NCH):
                emit_C_chunk(tc, pools, consts, st, k, skip)
            emit_D(tc, pools, consts, st, skip)
        prev = st


def _build_nc(reps: int = 1, skip=frozenset()):
    nc = bacc.Bacc("TRN2", target_bir_lowering=False, debug=False,
                   num_devices=NCORES)
    x_d = nc.dram_tensor("x", [BLOC, C, HW], BF16, kind="ExternalInput").ap()
    w1t_d = nc.dram_tensor("w1t", [C, R], BF16, kind="ExternalInput").ap()
    w2t_d = nc.dram_tensor("w2t", [R, C], BF16, kind="ExternalInput").ap()
    bands_d = nc.dram_tensor("bands", [14, W, W], BF16,
                             kind="ExternalInput").ap()
    oh_d = nc.dram_tensor("oh", [P, 2 * W], BF16, kind="ExternalInput").ap()
    out_d = nc.dram_tensor("out", [BLOC, C, HW], BF16,
                           kind="ExternalOutput").ap()
    with tile.TileContext(nc) as tc:
        with ExitStack() as ctx:
            pools = make_pools(ctx, tc)
            consts = load_consts(tc, pools, w1t_d, w2t_d, bands_d, oh_d)
            emit_pipeline(tc, pools, consts, out_d, x_d, reps * BLOC,
                          skip=skip)
    nc.compile()
    return nc


_NC_CACHE = None


def _get_nc():
    global _NC_CACHE
    if _NC_CACHE is None:
        _NC_CACHE = _build_nc()
    return _NC_CACHE


def build_bands(w_conv):
    """[14, H, H] band matrices for the [h, w] conv formulation:
    bands[c*7+j][h, h'] = w_conv[0, c, h-h'+3, j]; avg channel folded 1/C."""
    w_conv = np.asarray(w_conv, np.float32)
    bands = np.zeros((2, 7, H, H), np.float32)
    for c in range(2):
        for j in range(7):
            for i in range(7):
                bands[c, j] += w_conv[0, c, i, j] * np.eye(H, k=3 - i,
                                                           dtype=np.float32)
    bands[1] /= C
    return bands.reshape(14, H, H)


def build_onehot():
    oh = np.zeros((P, 2 * W), np.float32)
    oh[:, W] = 1.0
    return oh


def make_in_maps(x, w_fc1, w_fc2, w_conv):
    bf = ml_dtypes.bfloat16
    x = np.ascontiguousarray(np.asarray(x)).astype(bf)
    w1t = np.ascontiguousarray(np.asarray(w_fc1, np.float32).T).astype(bf)
    w2t = np.ascontiguousarray(np.asarray(w_fc2, np.float32).T).astype(bf)
    bands = build_bands(w_conv).astype(bf)
    oh = build_onehot().astype(bf)
    xr = x.reshape(NCORES, BLOC, C, HW)
    return [{"x": np.ascontiguousarray(xr[i]), "w1t": w1t, "w2t": w2t,
             "bands": bands, "oh": oh} for i in range(NCORES)]


def kernel(x, w_fc1, w_fc2, w_conv):
    nc = _get_nc()
    in_maps = make_in_maps(x, w_fc1, w_fc2, w_conv)
    res = run_bass_kernel_spmd(nc, in_maps, list(range(NCORES)))
    out = np.stack([np.asarray(res.results[i]["out"]) for i in range(NCORES)])
    return out.reshape(B, C, H, W).astype(np.float32)
